# revision 59
# baseline (speedup 1.0000x reference)
"""MLA-style attention (shared latent KV head, attention sink, partial RoPE,
low-rank Q and grouped low-rank output projection) on 8 TRN2 NeuronCores.

Sharding: 64 query heads split 8 per core (tensor parallel on wq_b rows /
wo_a groups); latent KV path seq-sharded then all-gathered; final wo_b
matmul computed as per-core partial products summed on the host.

Structure (vs the original baseline):
- stage A computes the local qr/kv seq tile and all-gathers qrT+kv
- max-free softmax: logits bounded by sqrt(HD) (q,kv rms-normed), so no
  row-max pass; the q-RMS scale is folded into the softmax exp's
  per-partition scale AP (which also makes the qr RMS norm a no-op,
  so it is skipped)
- all rsqrt computations run as Newton iterations on DVE, so the ACT
  engine never switches activation tables (exp set only)
- RoPE batched across all seq tiles; PE transposes for q/p/o; xbar DMA
  transposes for qrT/kvT
- stage E streams wo_b in eighths and writes output partials from PSUM
  via SBUF on the ACT DMA queue
- with reps>1 the body is software-pipelined: stage A of rep r is
  emitted between BC and E of rep r-1, so the all-gather of rep r
  overlaps stage E of rep r-1 (double-buffered qrT/kv interface)
"""

import numpy as np
import ml_dtypes

import concourse.bass as bass
import concourse.mybir as mybir
import concourse.tile as tile
from concourse import bacc
from concourse.bass_utils import run_bass_kernel_spmd
from concourse.masks import make_identity, make_causal_mask

BF16 = mybir.dt.bfloat16
F32 = mybir.dt.float32
AX = mybir.AxisListType
ALU = mybir.AluOpType
ACTF = mybir.ActivationFunctionType

NPBF16 = ml_dtypes.bfloat16

# problem dims (hardcoded; kernel.py must be self-contained)
D, NH, HD, RD, QLR, OLR, OG = 4096, 64, 512, 64, 1024, 1024, 8
S = 1024
NCORES = 8
HPC = NH // NCORES  # query heads per core
EPS = 1e-6
P = 128


class Cfg:
    def __init__(self, s=S, d=D, qlr=QLR, hpc=HPC, olr=OLR, outd=D):
        assert s % P == 0 and d % P == 0 and qlr % 512 == 0 and olr % 512 == 0
        assert outd % 512 == 0
        self.s, self.d, self.qlr, self.hpc, self.olr, self.outd = (
            s, d, qlr, hpc, olr, outd)
        self.sc = s // P        # seq tiles
        self.dc = d // P        # model-dim chunks (contraction for qr/kv)
        self.qc = qlr // P      # q_lora chunks
        self.hc = HD // P       # head-dim chunks (4)
        self.f = hpc * HD       # per-core attention output feature dim
        self.fc = self.f // P   # feature chunks for wo_a contraction
        self.oc = olr // P      # olr chunks (contraction for wo_b)
        self.nc_out = outd // 512  # output D chunks


def _rope_tail(nc, pool, dst, cos_ap, sin_ap, inverse, tag, bufs=1):
    """Partial RoPE on dst[..., HD-RD:HD] in place.

    dst: [128, HD] or [128, sc, HD] bf16; cos/sin: matching [128, RD//2] or
    [128, sc, RD//2] f32."""
    if len(dst.shape) == 3:
        tail = dst[:, :, HD - RD:HD].rearrange("p s (a two) -> p s a two",
                                               two=2)
        x1 = tail[:, :, :, 0]
        x2 = tail[:, :, :, 1]
        tshape = [P, dst.shape[1], RD // 2]
    else:
        tail = dst[:, HD - RD:HD].rearrange("p (a two) -> p a two", two=2)
        x1 = tail[:, :, 0]
        x2 = tail[:, :, 1]
        tshape = [P, RD // 2]
    nd = len(tshape)
    t1 = pool.tile(tshape, F32, tag=f"rope{nd}_1", bufs=bufs)
    t2 = pool.tile(tshape, F32, tag=f"rope{nd}_2", bufs=bufs)
    t3 = pool.tile(tshape, F32, tag=f"rope{nd}_3", bufs=bufs)
    nc.vector.tensor_mul(t1[:], x1, cos_ap)
    nc.vector.tensor_mul(t2[:], x2, sin_ap)
    nc.vector.tensor_mul(t3[:], x1, sin_ap)
    if not inverse:
        # x1 = x1*c - x2*s ; x2 = x1*s + x2*c  (t2 reused for x2*c)
        nc.vector.tensor_sub(x1, t1[:], t2[:])
        nc.vector.tensor_mul(t2[:], x2, cos_ap)
        nc.vector.tensor_add(x2, t3[:], t2[:])
    else:
        # x1 = x1*c + x2*s ; x2 = x2*c - x1*s
        nc.vector.tensor_add(x1, t1[:], t2[:])
        nc.vector.tensor_mul(t2[:], x2, cos_ap)
        nc.vector.tensor_sub(x2, t2[:], t3[:])


def _rsqrt_dve(nc, pool, out, ssq, mean_scale, post_scale, tag):
    """out = post_scale / sqrt(ssq * mean_scale + EPS), entirely on DVE
    (Newton iterations; avoids ACT Ln so the activation table never has to
    switch away from the exp set).  ssq/out: [P, n] f32."""
    shape = list(ssq.shape)
    m = pool.tile(shape, F32, tag=tag + "_m", bufs=1)
    r = pool.tile(shape, F32, tag=tag + "_r", bufs=1)
    t = pool.tile(shape, F32, tag=tag + "_t", bufs=1)
    nc.vector.tensor_scalar(m[:], ssq, float(mean_scale), float(EPS),
                            ALU.mult, ALU.add)
    # clamped linear seed r0 = max(1.5 - 0.5 m, 0.2)
    nc.vector.tensor_scalar(r[:], m[:], -0.5, 1.5, ALU.mult, ALU.add)
    nc.vector.tensor_scalar_max(r[:], r[:], 0.2)
    for _ in range(4):
        nc.vector.tensor_mul(t[:], r[:], r[:])
        nc.vector.tensor_mul(t[:], t[:], m[:])
        nc.vector.tensor_scalar(t[:], t[:], -0.5, 1.5, ALU.mult, ALU.add)
        nc.vector.tensor_mul(r[:], r[:], t[:])
    nc.vector.tensor_scalar_mul(out, r[:], float(post_scale))


class _Dram:
    """DRAM tensor handles."""

    def __init__(self, nc, cfg):
        self.xt = nc.dram_tensor("xtm", [P, cfg.dc, P], BF16,
                                 kind="ExternalInput").ap()
        self.cosm = nc.dram_tensor("cosm", [P, RD // 2], F32,
                                   kind="ExternalInput").ap()
        self.sinm = nc.dram_tensor("sinm", [P, RD // 2], F32,
                                   kind="ExternalInput").ap()
        self.wqa = nc.dram_tensor("wqa", [P, cfg.dc, cfg.qlr], BF16,
                                  kind="ExternalInput").ap()
        self.wkv = nc.dram_tensor("wkv", [P, cfg.dc, HD], BF16,
                                  kind="ExternalInput").ap()
        self.wqb = nc.dram_tensor("wqb", [P, cfg.qc, cfg.hpc * HD], BF16,
                                  kind="ExternalInput").ap()
        self.woa = nc.dram_tensor("woa", [P, cfg.fc, cfg.olr], BF16,
                                  kind="ExternalInput").ap()
        self.wob = nc.dram_tensor("wob", [P, cfg.oc, cfg.outd], BF16,
                                  kind="ExternalInput").ap()
        self.cos = nc.dram_tensor("coss", [P, cfg.sc, RD // 2], F32,
                                  kind="ExternalInput").ap()
        self.sin = nc.dram_tensor("sins", [P, cfg.sc, RD // 2], F32,
                                  kind="ExternalInput").ap()
        self.sinkexp = nc.dram_tensor("sinkexp", [P, cfg.hpc], F32,
                                      kind="ExternalInput").ap()
        self.out = nc.dram_tensor("out", [cfg.sc, P, cfg.outd], F32,
                                  kind="ExternalOutput").ap()


class _Glob:
    """Per-program persistent SBUF tiles (loaded once)."""

    def __init__(self, nc, gp, cfg, dram):
        self.ident = gp.tile([P, P], BF16)
        make_identity(nc, self.ident[:])
        self.cmask = gp.tile([P, P], F32)
        make_causal_mask(nc, self.cmask[:], mask_val=-1e10)
        self.sinkexp = gp.tile([P, cfg.hpc], F32)
        nc.gpsimd.dma_start(self.sinkexp[:], dram.sinkexp)
        self.cos = gp.tile([P, cfg.sc, RD // 2], F32)
        nc.gpsimd.dma_start(self.cos[:], dram.cos)
        self.sin = gp.tile([P, cfg.sc, RD // 2], F32)
        nc.gpsimd.dma_start(self.sin[:], dram.sin)
        self.cosm = gp.tile([P, RD // 2], F32)
        nc.gpsimd.dma_start(self.cosm[:], dram.cosm)
        self.sinm = gp.tile([P, RD // 2], F32)
        nc.gpsimd.dma_start(self.sinm[:], dram.sinm)
        # single-buffered latent-KV transpose (rebuilt at each BC start)
        self.kvT = gp.tile([P, cfg.hc, cfg.s], BF16)


def _stage_a(nc, tc, cfg, gp, g, dram):
    """Local qr + kv tile, all-gather.  Returns (qrT_sb, kv_sb) interface
    tiles (tag-rotated, bufs=2, so the next call uses the other buffer)."""
    sc, dc, qc = cfg.sc, cfg.dc, cfg.qc
    qrT_sb = gp.tile([P, qc, cfg.s], BF16, tag="qrT", bufs=2)
    kv_sb = gp.tile([P, sc, HD], BF16, tag="kvg", bufs=2)

    with tc.tile_pool(name="stA", bufs=2) as pa, \
         tc.tile_pool(name="stAw", bufs=2) as paw, \
         tc.tile_pool(name="psA", bufs=1, space="PSUM") as psa:
        xt_i = paw.tile([P, dc, P], BF16, tag="xt", bufs=1)
        nc.sync.dma_start(xt_i[:], dram.xt)
        qr_ps = psa.tile([P, cfg.qlr], F32, tag="qr", bufs=1)
        kv_ps = psa.tile([P, HD], F32, tag="kv", bufs=1)
        ngr = 8
        gsz = dc // ngr
        for gi in range(ngr):
            wqa_g = pa.tile([P, gsz, cfg.qlr], BF16, tag="wqa")
            nc.sync.dma_start(wqa_g[:],
                              dram.wqa[:, gi * gsz:(gi + 1) * gsz, :])
            wkv_g = pa.tile([P, gsz, HD], BF16, tag="wkv")
            nc.sync.dma_start(wkv_g[:],
                              dram.wkv[:, gi * gsz:(gi + 1) * gsz, :])
            for kk in range(gsz):
                k = gi * gsz + kk
                st, sp = k == 0, k == dc - 1
                for n2 in range(cfg.qlr // 512):
                    nc.tensor.matmul(
                        qr_ps[:, n2 * 512:(n2 + 1) * 512],
                        xt_i[:, k, :],
                        wqa_g[:, kk, n2 * 512:(n2 + 1) * 512],
                        start=st, stop=sp)
                nc.tensor.matmul(kv_ps[:], xt_i[:, k, :],
                                 wkv_g[:, kk, :], start=st, stop=sp)

        # qr epilogue: cast + transpose only.  The qr RMS norm is skipped:
        # the per-head q-RMS scale (folded into the softmax exp) cancels
        # any per-row scaling of qr, so normalizing here is a no-op up to
        # eps=1e-6.
        qrn = paw.tile([P, cfg.qlr], BF16, tag="qrn")
        nc.any.tensor_copy(qrn[:], qr_ps[:])
        qrT_loc = paw.tile([P, qc, P], BF16, tag="qrT_loc", bufs=1)
        nc.sync.dma_start_transpose(qrT_loc[:], qrn[:])

        # local kv epilogue (kv_norm_w folded into wkv on the host)
        sqk = paw.tile([P, HD], F32, tag="sqk", bufs=1)
        ssqk = paw.tile([P, 1], F32, tag="ssqk")
        nc.scalar.activation(sqk[:], kv_ps[:], ACTF.Square,
                             accum_out=ssqk[:])
        rinvk = paw.tile([P, 1], F32, tag="rinvk")
        _rsqrt_dve(nc, paw, rinvk[:], ssqk[:], 1.0 / HD, 1.0, tag="rkvn")
        kv_loc = paw.tile([P, HD], BF16, tag="kv_loc", bufs=1)
        nc.scalar.mul(kv_loc[:], kv_ps[:], rinvk[:])
        _rope_tail(nc, paw, kv_loc[:], g.cosm[:], g.sinm[:],
                   False, tag="rkv", bufs=2)

        # pack qrT + local kv into DRAM and all-gather
        gw = qc * P + HD
        with tc.tile_pool(name="ccdram", bufs=1, space="DRAM") as ccd:
            gin = ccd.tile([P, gw], BF16)
            gout = ccd.tile([NCORES, P, gw], BF16, addr_space="Shared")
            nc.sync.dma_start(gin[:, 0:qc * P],
                              qrT_loc[:].rearrange("p c s -> p (c s)"))
            nc.sync.dma_start(gin[:, qc * P:gw], kv_loc[:])
            nc.gpsimd.collective_compute(
                "AllGather", ALU.bypass,
                replica_groups=[list(range(NCORES))],
                ins=[gin[:]], outs=[gout[:]])
            for j in range(NCORES):
                nc.sync.dma_start(
                    qrT_sb[:, :, j * P:(j + 1) * P],
                    gout[j, :, 0:qc * P].rearrange("p (c s) -> p c s", c=qc))
                nc.sync.dma_start(kv_sb[:, j, :], gout[j, :, qc * P:gw])
    return qrT_sb, kv_sb


def _bc_preload(nc, cfg, pew, dram):
    """Head 0/1 weight loads, emitted before the next stage A so they are
    not head-of-line blocked behind its collective on the Pool queue."""
    hc, qc = cfg.hc, cfg.qc
    pre = []
    for h in range(2):
        wqb_h = pew.tile([P, qc, HD], BF16, tag="wqb_h", bufs=2)
        nc.gpsimd.dma_start(wqb_h[:], dram.wqb[:, :, h * HD:(h + 1) * HD])
        woa_h = None
        if h == 0:
            woa_h = pew.tile([P, hc, cfg.olr], BF16, tag="woa_h", bufs=1)
            nc.gpsimd.dma_start(woa_h[:],
                                dram.woa[:, h * hc:(h + 1) * hc, :])
        pre.append((wqb_h, woa_h))
    return pre


def _stage_bc(nc, tc, cfg, g, ogp, pew, iface, dram, pre):
    """Per-head q proj + attention + wo_a partials.  Returns (ogT, wob_qs)
    for stage E."""
    sc, qc, hc = cfg.sc, cfg.qc, cfg.hc
    qrT_sb, kv_sb = iface
    kvT_sb = g.kvT
    ident, cmask = g.ident, g.cmask
    cos_sb, sin_sb, sinkexp_sb = g.cos, g.sin, g.sinkexp
    s_chunks = [(a, min(512, cfg.s - a)) for a in range(0, cfg.s, 512)]

    og_acc = ogp.tile([P, cfg.oc, cfg.s], F32, tag="og", bufs=1)
    ogT_sb = ogp.tile([P, cfg.oc, cfg.s], BF16, tag="ogT", bufs=1)
    # prefetch the first wo_b eighths (bufs=3 so these never WAR-wait);
    # the rest stream inside stage E right before use
    wob_qs = []
    for e in range(3):
        wob_q = pew.tile([P, cfg.oc, 512], BF16, tag="wobq", bufs=3)
        nc.scalar.dma_start(wob_q[:],
                            dram.wob[:, :, e * 512:(e + 1) * 512])
        wob_qs.append(wob_q)

    # kvT derived from the gathered kv (xbar transpose)
    for j in range(sc):
        nc.sync.dma_start_transpose(
            kvT_sb[:, :, j * P:(j + 1) * P], kv_sb[:, j, :])

    with tc.tile_pool(name="stBC", bufs=1) as pb, \
         tc.tile_pool(name="stBCw", bufs=2) as pbw, \
         tc.tile_pool(name="psQ", bufs=1, space="PSUM") as psq, \
         tc.tile_pool(name="psS", bufs=1, space="PSUM") as pss, \
         tc.tile_pool(name="psT", bufs=1, space="PSUM") as pst, \
         tc.tile_pool(name="psO", bufs=1, space="PSUM") as pso, \
         tc.tile_pool(name="psD", bufs=1, space="PSUM") as psd:
        for h in range(cfg.hpc):
            if h < 2 and pre[h][1] is not None:
                wqb_h, woa_h = pre[h]
            elif h < 2:
                wqb_h = pre[h][0]
                woa_h = pew.tile([P, hc, cfg.olr], BF16, tag="woa_h",
                                 bufs=1)
                nc.gpsimd.dma_start(
                    woa_h[:], dram.woa[:, h * hc:(h + 1) * hc, :])
            else:
                woa_h = pew.tile([P, hc, cfg.olr], BF16, tag="woa_h",
                                 bufs=1)
                nc.gpsimd.dma_start(
                    woa_h[:], dram.woa[:, h * hc:(h + 1) * hc, :])
                wqb_h = pew.tile([P, qc, HD], BF16, tag="wqb_h", bufs=2)
                nc.gpsimd.dma_start(
                    wqb_h[:], dram.wqb[:, :, h * HD:(h + 1) * HD])
            qT_sb = pbw.tile([P, hc, cfg.s], BF16, tag="qT", bufs=1)
            # ---- q projection (q left unnormalized; the RMS scale is
            # folded into the softmax exp below) ----
            q8 = pbw.tile([P, sc, HD], BF16, tag="q8", bufs=1)
            ssq8 = pbw.tile([P, sc], F32, tag="ssq8")
            for i in range(sc):
                q_ps = psq.tile([P, HD], F32, tag="q", bufs=2)
                for c in range(qc):
                    nc.tensor.matmul(
                        q_ps[:], qrT_sb[:, c, i * P:(i + 1) * P],
                        wqb_h[:, c, :], start=(c == 0), stop=(c == qc - 1))
                nc.any.tensor_copy(q8[:, i, :], q_ps[:])
                sqq = pbw.tile([P, HD], F32, tag="sqq", bufs=1)
                nc.scalar.activation(sqq[:], q8[:, i, :], ACTF.Square,
                                     accum_out=ssq8[:, i:i + 1])
            rinv8 = pbw.tile([P, sc], F32, tag="rinv8")
            _rsqrt_dve(nc, pbw, rinv8[:], ssq8[:], 1.0 / HD,
                       HD ** -0.5, tag="rq8")
            _rope_tail(nc, pbw, q8[:], cos_sb[:], sin_sb[:],
                       False, tag="rq")
            for i in range(sc):
                tpq = pst.tile([P, 512], BF16, tag="t", bufs=1)
                for c in range(hc):
                    nc.tensor.transpose(tpq[:, c * P:(c + 1) * P],
                                        q8[:, i, c * P:(c + 1) * P],
                                        ident[:])
                nc.any.tensor_copy(
                    qT_sb[:, :, i * P:(i + 1) * P],
                    tpq[:].rearrange("p (c s) -> p c s", c=hc))

            # ---- attention for head h (max-free softmax) ----
            o8 = pbw.tile([P, sc, HD], BF16, tag="o8", bufs=1)
            for i in range(sc):
                w_all = (i + 1) * P
                nch = (w_all + 511) // 512
                s_ps = []
                for ci in range(nch):
                    wci = min(512, w_all - ci * 512)
                    s_ps.append((pss.tile([P, 512], F32, tag="s",
                                          bufs=2, name="s_ps"), wci))
                for k in range(hc):
                    for ci in range(nch):
                        tile_ps, wci = s_ps[ci]
                        nc.tensor.matmul(
                            tile_ps[:, :wci],
                            qT_sb[:, k, i * P:(i + 1) * P],
                            kvT_sb[:, k, ci * 512:ci * 512 + wci],
                            start=(k == 0), stop=(k == hc - 1))
                # causal mask on the diagonal block
                dps, dw = s_ps[-1]
                dcol = (w_all - P) - (nch - 1) * 512
                nc.vector.tensor_add(dps[:, dcol:dcol + P],
                                     dps[:, dcol:dcol + P], cmask[:])
                # exp with the q-RMS scale folded in; row sums accumulate
                # per chunk (no max subtraction: |logit| <= sqrt(HD) so
                # exp stays in f32 range)
                p_sb = pbw.tile([P, cfg.s], BF16, tag="p")
                l0 = pb.tile([P, 2], F32, tag="l0", bufs=2)
                for ci in range(nch):
                    tile_ps, wci = s_ps[ci]
                    nc.scalar.activation(
                        p_sb[:, ci * 512:ci * 512 + wci],
                        tile_ps[:, :wci], ACTF.Exp,
                        scale=rinv8[:, i:i + 1],
                        accum_out=l0[:, ci:ci + 1])
                lsum = pb.tile([P, 1], F32, tag="lsum", bufs=2)
                if nch == 1:
                    nc.vector.tensor_add(lsum[:], l0[:, 0:1],
                                         sinkexp_sb[:, h:h + 1])
                else:
                    nc.vector.tensor_add(lsum[:], l0[:, 0:1], l0[:, 1:2])
                    nc.vector.tensor_add(lsum[:], lsum[:],
                                         sinkexp_sb[:, h:h + 1])
                linv = pb.tile([P, 1], F32, tag="linv", bufs=2)
                nc.vector.reciprocal(linv[:], lsum[:])
                # transpose p
                pT_sb = pbw.tile([P, cfg.s], BF16, tag="pT")
                for gi in range((i + 1 + 3) // 4):
                    jn = min(4, (i + 1) - gi * 4)
                    tpp = pst.tile([P, 512], BF16, tag="t", bufs=1)
                    for j4 in range(jn):
                        j = gi * 4 + j4
                        nc.tensor.transpose(
                            tpp[:, j4 * P:(j4 + 1) * P],
                            p_sb[:, j * P:(j + 1) * P], ident[:])
                    nc.any.tensor_copy(
                        pT_sb[:, gi * 512:gi * 512 + jn * P],
                        tpp[:, :jn * P])
                # o = p^T-weighted sum of kv rows
                o_ps = pso.tile([P, HD], F32, tag="o", bufs=1)
                for j in range(i + 1):
                    nc.tensor.matmul(o_ps[:], pT_sb[:, j * P:(j + 1) * P],
                                     kv_sb[:, j, :],
                                     start=(j == 0), stop=(j == i))
                # normalize into o8 (batched inv-rope later)
                nc.vector.tensor_scalar_mul(o8[:, i, :], o_ps[:], linv[:])

            _rope_tail(nc, pbw, o8[:], cos_sb[:], sin_sb[:], True, tag="ro")
            oT_h = pbw.tile([P, hc, cfg.s], BF16, tag="oT_h", bufs=1)
            for i in range(sc):
                tpo = pst.tile([P, 512], BF16, tag="t", bufs=1)
                for c in range(hc):
                    nc.tensor.transpose(tpo[:, c * P:(c + 1) * P],
                                        o8[:, i, c * P:(c + 1) * P],
                                        ident[:])
                nc.any.tensor_copy(
                    oT_h[:, :, i * P:(i + 1) * P],
                    tpo[:].rearrange("p (c s) -> p c s", c=hc))

            # ---- wo_a partial for this head, into f32 og_acc ----
            for m in range(cfg.oc):
                d_ps = []
                for n2 in range(len(s_chunks)):
                    d_ps.append(psd.tile([P, 512], F32, tag=f"d{n2}",
                                         bufs=1, name="d_ps"))
                for kk in range(hc):
                    for n2, (a, w) in enumerate(s_chunks):
                        nc.tensor.matmul(
                            d_ps[n2][:, :w],
                            woa_h[:, kk, m * P:(m + 1) * P],
                            oT_h[:, kk, a:a + w],
                            start=(kk == 0), stop=(kk == hc - 1))
                for n2, (a, w) in enumerate(s_chunks):
                    if h == 0:
                        nc.vector.tensor_copy(og_acc[:, m, a:a + w],
                                              d_ps[n2][:, :w])
                    elif h == cfg.hpc - 1:
                        # final add writes bf16 ogT directly
                        nc.vector.tensor_add(ogT_sb[:, m, a:a + w],
                                             og_acc[:, m, a:a + w],
                                             d_ps[n2][:, :w])
                    else:
                        nc.vector.tensor_add(og_acc[:, m, a:a + w],
                                             og_acc[:, m, a:a + w],
                                             d_ps[n2][:, :w])
    return ogT_sb, wob_qs


def _stage_e(nc, tc, cfg, pew, ogT_sb, wob_qs, dram):
    """Final wo_b partial matmul, streaming output to DRAM."""
    sc = cfg.sc
    with tc.tile_pool(name="psE", bufs=1, space="PSUM") as pse:
        for quarter in range(cfg.nc_out):
            if quarter < len(wob_qs):
                wob_q = wob_qs[quarter]
            else:
                wob_q = pew.tile([P, cfg.oc, 512], BF16, tag="wobq",
                                 bufs=3)
                nc.scalar.dma_start(
                    wob_q[:],
                    dram.wob[:, :, quarter * 512:(quarter + 1) * 512])
            for m in range(sc):
                out_ps = pse.tile([P, 512], F32, tag="out", bufs=8,
                                  name="out_ps")
                for k in range(cfg.oc):
                    nc.tensor.matmul(
                        out_ps[:], ogT_sb[:, k, m * P:(m + 1) * P],
                        wob_q[:, k, :], start=(k == 0),
                        stop=(k == cfg.oc - 1))
                o_out = pew.tile([P, 512], F32, tag="oo", bufs=3)
                nc.vector.tensor_copy(o_out[:], out_ps[:])
                nc.scalar.dma_start(
                    dram.out[m, :, quarter * 512:(quarter + 1) * 512],
                    o_out[:])


def build_program(cfg: Cfg, debug=False, reps=1):
    nc = bacc.Bacc("TRN2", debug=False, num_devices=NCORES)
    dram = _Dram(nc, cfg)

    with tile.TileContext(nc) as tc:
        with tc.tile_pool(name="glob", bufs=1) as gp, \
             tc.tile_pool(name="og", bufs=1) as ogp, \
             tc.tile_pool(name="stEw", bufs=2) as pew:
            g = _Glob(nc, gp, cfg, dram)
            # software pipeline: stage A of rep r runs between BC and E of
            # rep r-1, hiding the all-gather under stage E
            prev = _stage_a(nc, tc, cfg, gp, g, dram)
            for _ in range(reps - 1):
                pre = _bc_preload(nc, cfg, pew, dram)
                nxt = _stage_a(nc, tc, cfg, gp, g, dram)
                bce = _stage_bc(nc, tc, cfg, g, ogp, pew, prev, dram, pre)
                _stage_e(nc, tc, cfg, pew, bce[0], bce[1], dram)
                prev = nxt
            pre = _bc_preload(nc, cfg, pew, dram)
            bce = _stage_bc(nc, tc, cfg, g, ogp, pew, prev, dram, pre)
            _stage_e(nc, tc, cfg, pew, bce[0], bce[1], dram)

    nc.compile()
    return nc


# ---------------------------------------------------------------------------
# host side
# ---------------------------------------------------------------------------

def _pack_kt(w, n_rows, n_cols):
    """Pack W (given as [n_cols, n_rows] np array) into [128, n_rows/128,
    n_cols] = W.T tiled with the contraction dim on partitions."""
    wt = np.ascontiguousarray(w.T)  # [n_rows, n_cols]
    return np.ascontiguousarray(
        wt.reshape(n_rows // P, P, n_cols).transpose(1, 0, 2))


def prepare_inmaps(inputs, cfg: Cfg):
    bf = NPBF16
    x = np.asarray(inputs["x"], dtype=bf).reshape(cfg.s, cfg.d)
    xt = np.ascontiguousarray(
        x.T.reshape(cfg.dc, P, cfg.sc, P).transpose(2, 1, 0, 3))

    wq_a = np.asarray(inputs["wq_a"], dtype=bf)
    wqa = _pack_kt(wq_a, cfg.d, cfg.qlr)

    kv_norm_w = np.asarray(inputs["kv_norm_w"], dtype=np.float32)
    wkv_f = (np.asarray(inputs["wkv"], dtype=bf).astype(np.float32)
             * kv_norm_w[:, None]).astype(bf)
    wkv = _pack_kt(wkv_f, cfg.d, HD)

    q_norm_w = np.asarray(inputs["q_norm_w"], dtype=np.float32)
    wq_b = np.asarray(inputs["wq_b"], dtype=bf).astype(np.float32)
    wq_b = (wq_b * q_norm_w[None, :]).astype(bf)  # fold q_norm into wq_b

    cos = np.asarray(inputs["cos"], dtype=np.float32)
    sin = np.asarray(inputs["sin"], dtype=np.float32)
    cos_p = np.ascontiguousarray(
        cos.reshape(cfg.sc, P, RD // 2).transpose(1, 0, 2))
    sin_p = np.ascontiguousarray(
        sin.reshape(cfg.sc, P, RD // 2).transpose(1, 0, 2))

    wo_a = np.asarray(inputs["wo_a"], dtype=bf)  # [OG*OLR, F]
    wo_b = np.asarray(inputs["wo_b"], dtype=bf)  # [D, OG*OLR]
    sink = np.asarray(inputs["attn_sink"], dtype=np.float32)

    in_maps = []
    for c in range(NCORES):
        h0 = c * cfg.hpc
        wqb_c = wq_b[h0 * HD:(h0 + cfg.hpc) * HD, :]  # [hpc*HD, qlr]
        woa_c = wo_a[c * cfg.olr:(c + 1) * cfg.olr, :]  # [olr, F]
        wob_c = wo_b[:, c * cfg.olr:(c + 1) * cfg.olr]  # [outd, olr]
        sinkexp_c = np.exp(sink[h0:h0 + cfg.hpc])
        in_maps.append({
            "xtm": np.ascontiguousarray(xt[c]),
            "cosm": np.ascontiguousarray(cos_p[:, c, :]),
            "sinm": np.ascontiguousarray(sin_p[:, c, :]),
            "wqa": wqa,
            "wkv": wkv,
            "wqb": _pack_kt(wqb_c, cfg.qlr, cfg.hpc * HD),
            "woa": _pack_kt(woa_c, cfg.f, cfg.olr),
            "wob": _pack_kt(wob_c, cfg.olr, cfg.outd),
            "coss": cos_p,
            "sins": sin_p,
            "sinkexp": np.ascontiguousarray(
                np.broadcast_to(sinkexp_c, (P, cfg.hpc))).astype(np.float32),
        })
    return in_maps


_CACHE = {}


def _get_program():
    if "nc" not in _CACHE:
        _CACHE["nc"] = build_program(Cfg())
    return _CACHE["nc"]


def run(inputs, trace=False):
    """Returns (output [1,S,D] bf16, BassKernelResults)."""
    cfg = Cfg()
    nc = _get_program()
    in_maps = prepare_inmaps(inputs, cfg)
    res = run_bass_kernel_spmd(nc, in_maps, core_ids=list(range(NCORES)),
                               trace=trace)
    acc = np.zeros((cfg.s, cfg.outd), np.float32)
    for r in res.results:
        acc += r["out"].reshape(cfg.s, cfg.outd)
    out = acc.astype(NPBF16).reshape(1, cfg.s, cfg.outd)
    return out, res


def kernel(**inputs) -> np.ndarray:
    out, _ = run(inputs)
    return out


# revision 62
# speedup vs baseline: 1.2933x; 1.2933x over previous
"""MLA-style attention (shared latent KV head, attention sink, partial RoPE,
low-rank Q and grouped low-rank output projection) on 8 TRN2 NeuronCores.

Sharding: 64 query heads split 8 per core (tensor parallel on wq_b rows /
wo_a groups); latent KV path seq-sharded then all-gathered; final wo_b
matmul computed as per-core partial products summed on the host.

Structure (vs the original baseline):
- stage A computes the local qr/kv seq tile and all-gathers qrT+kv
- max-free softmax: logits bounded by sqrt(HD) (q,kv rms-normed), so no
  row-max pass; the q-RMS scale is folded into the softmax exp's
  per-partition scale AP (which also makes the qr RMS norm a no-op,
  so it is skipped)
- all rsqrt computations run as Newton iterations on DVE, so the ACT
  engine never switches activation tables (exp set only)
- RoPE batched across all seq tiles; PE transposes for q/p/o; xbar DMA
  transposes for qrT/kvT
- stage E streams wo_b in eighths and writes output partials from PSUM
  via SBUF on the ACT DMA queue
- with reps>1 the body is software-pipelined: stage A of rep r is
  emitted between BC and E of rep r-1, so the all-gather of rep r
  overlaps stage E of rep r-1 (double-buffered qrT/kv interface)
"""

import numpy as np
import ml_dtypes

import concourse.bass as bass
import concourse.mybir as mybir
import concourse.tile as tile
from concourse import bacc
from concourse.bass_utils import run_bass_kernel_spmd
from concourse.masks import make_identity, make_causal_mask

BF16 = mybir.dt.bfloat16
F32 = mybir.dt.float32
AX = mybir.AxisListType
ALU = mybir.AluOpType
ACTF = mybir.ActivationFunctionType

NPBF16 = ml_dtypes.bfloat16

# problem dims (hardcoded; kernel.py must be self-contained)
D, NH, HD, RD, QLR, OLR, OG = 4096, 64, 512, 64, 1024, 1024, 8
S = 1024
NCORES = 8
HPC = NH // NCORES  # query heads per core
EPS = 1e-6
P = 128


class Cfg:
    def __init__(self, s=S, d=D, qlr=QLR, hpc=HPC, olr=OLR, outd=D):
        assert s % P == 0 and d % P == 0 and qlr % 512 == 0 and olr % 512 == 0
        assert outd % 512 == 0
        self.s, self.d, self.qlr, self.hpc, self.olr, self.outd = (
            s, d, qlr, hpc, olr, outd)
        self.sc = s // P        # seq tiles
        self.dc = d // P        # model-dim chunks (contraction for qr/kv)
        self.qc = qlr // P      # q_lora chunks
        self.hc = HD // P       # head-dim chunks (4)
        self.f = hpc * HD       # per-core attention output feature dim
        self.fc = self.f // P   # feature chunks for wo_a contraction
        self.oc = olr // P      # olr chunks (contraction for wo_b)
        self.nc_out = outd // 512  # output D chunks


def _rope_tail(nc, pool, dst, cos_ap, sin_ap, inverse, tag, bufs=1):
    """Partial RoPE on dst[..., HD-RD:HD] in place.

    dst: [128, HD] or [128, sc, HD] bf16; cos/sin: matching [128, RD//2] or
    [128, sc, RD//2] f32."""
    if len(dst.shape) == 3:
        tail = dst[:, :, HD - RD:HD].rearrange("p s (a two) -> p s a two",
                                               two=2)
        x1 = tail[:, :, :, 0]
        x2 = tail[:, :, :, 1]
        tshape = [P, dst.shape[1], RD // 2]
    else:
        tail = dst[:, HD - RD:HD].rearrange("p (a two) -> p a two", two=2)
        x1 = tail[:, :, 0]
        x2 = tail[:, :, 1]
        tshape = [P, RD // 2]
    nd = len(tshape)
    t1 = pool.tile(tshape, F32, tag=f"rope{nd}_1", bufs=bufs)
    t2 = pool.tile(tshape, F32, tag=f"rope{nd}_2", bufs=bufs)
    t3 = pool.tile(tshape, F32, tag=f"rope{nd}_3", bufs=bufs)
    nc.vector.tensor_mul(t1[:], x1, cos_ap)
    nc.vector.tensor_mul(t2[:], x2, sin_ap)
    nc.vector.tensor_mul(t3[:], x1, sin_ap)
    if not inverse:
        # x1 = x1*c - x2*s ; x2 = x1*s + x2*c  (t2 reused for x2*c)
        nc.vector.tensor_sub(x1, t1[:], t2[:])
        nc.vector.tensor_mul(t2[:], x2, cos_ap)
        nc.vector.tensor_add(x2, t3[:], t2[:])
    else:
        # x1 = x1*c + x2*s ; x2 = x2*c - x1*s
        nc.vector.tensor_add(x1, t1[:], t2[:])
        nc.vector.tensor_mul(t2[:], x2, cos_ap)
        nc.vector.tensor_sub(x2, t2[:], t3[:])


def _rsqrt_dve(nc, pool, out, ssq, mean_scale, post_scale, tag):
    """out = post_scale / sqrt(ssq * mean_scale + EPS), entirely on DVE
    (Newton iterations; avoids ACT Ln so the activation table never has to
    switch away from the exp set).  ssq/out: [P, n] f32."""
    shape = list(ssq.shape)
    m = pool.tile(shape, F32, tag=tag + "_m", bufs=1)
    r = pool.tile(shape, F32, tag=tag + "_r", bufs=1)
    t = pool.tile(shape, F32, tag=tag + "_t", bufs=1)
    nc.vector.tensor_scalar(m[:], ssq, float(mean_scale), float(EPS),
                            ALU.mult, ALU.add)
    # clamped linear seed r0 = max(1.5 - 0.5 m, 0.2)
    nc.vector.tensor_scalar(r[:], m[:], -0.5, 1.5, ALU.mult, ALU.add)
    nc.vector.tensor_scalar_max(r[:], r[:], 0.2)
    for _ in range(4):
        nc.vector.tensor_mul(t[:], r[:], r[:])
        nc.vector.tensor_mul(t[:], t[:], m[:])
        nc.vector.tensor_scalar(t[:], t[:], -0.5, 1.5, ALU.mult, ALU.add)
        nc.vector.tensor_mul(r[:], r[:], t[:])
    nc.vector.tensor_scalar_mul(out, r[:], float(post_scale))


class _Dram:
    """DRAM tensor handles."""

    def __init__(self, nc, cfg):
        self.xt = nc.dram_tensor("xtm", [P, cfg.dc, P], BF16,
                                 kind="ExternalInput").ap()
        self.cosm = nc.dram_tensor("cosm", [P, RD // 2], F32,
                                   kind="ExternalInput").ap()
        self.sinm = nc.dram_tensor("sinm", [P, RD // 2], F32,
                                   kind="ExternalInput").ap()
        self.wqa = nc.dram_tensor("wqa", [P, cfg.dc, cfg.qlr], BF16,
                                  kind="ExternalInput").ap()
        self.wkv = nc.dram_tensor("wkv", [P, cfg.dc, HD], BF16,
                                  kind="ExternalInput").ap()
        self.wqb = nc.dram_tensor("wqb", [P, cfg.qc, cfg.hpc * HD], BF16,
                                  kind="ExternalInput").ap()
        self.woa = nc.dram_tensor("woa", [P, cfg.fc, cfg.olr], BF16,
                                  kind="ExternalInput").ap()
        self.wob = nc.dram_tensor("wob", [P, cfg.oc, cfg.outd], BF16,
                                  kind="ExternalInput").ap()
        self.cos = nc.dram_tensor("coss", [P, cfg.sc, RD // 2], F32,
                                  kind="ExternalInput").ap()
        self.sin = nc.dram_tensor("sins", [P, cfg.sc, RD // 2], F32,
                                  kind="ExternalInput").ap()
        self.sinkexp = nc.dram_tensor("sinkexp", [P, cfg.hpc], F32,
                                      kind="ExternalInput").ap()
        self.out = nc.dram_tensor("out", [cfg.sc, P, cfg.outd], F32,
                                  kind="ExternalOutput").ap()


class _Glob:
    """Per-program persistent SBUF tiles (loaded once)."""

    def __init__(self, nc, gp, cfg, dram):
        self.ident = gp.tile([P, P], BF16)
        make_identity(nc, self.ident[:])
        self.cmask = gp.tile([P, P], F32)
        make_causal_mask(nc, self.cmask[:], mask_val=-1e10)
        self.sinkexp = gp.tile([P, cfg.hpc], F32)
        nc.gpsimd.dma_start(self.sinkexp[:], dram.sinkexp)
        self.cos = gp.tile([P, cfg.sc, RD // 2], F32)
        nc.gpsimd.dma_start(self.cos[:], dram.cos)
        self.sin = gp.tile([P, cfg.sc, RD // 2], F32)
        nc.gpsimd.dma_start(self.sin[:], dram.sin)
        self.cosm = gp.tile([P, RD // 2], F32)
        nc.gpsimd.dma_start(self.cosm[:], dram.cosm)
        self.sinm = gp.tile([P, RD // 2], F32)
        nc.gpsimd.dma_start(self.sinm[:], dram.sinm)
        # single-buffered latent-KV transpose (rebuilt at each BC start)
        self.kvT = gp.tile([P, cfg.hc, cfg.s], BF16)


def _stage_a(nc, tc, cfg, gp, g, dram):
    """Local qr + kv tile, all-gather.  Returns (qrT_sb, kv_sb) interface
    tiles (tag-rotated, bufs=2, so the next call uses the other buffer)."""
    sc, dc, qc = cfg.sc, cfg.dc, cfg.qc
    qrT_sb = gp.tile([P, qc, cfg.s], BF16, tag="qrT", bufs=2)
    kv_sb = gp.tile([P, sc, HD], BF16, tag="kvg", bufs=2)

    with tc.tile_pool(name="stA", bufs=2) as pa, \
         tc.tile_pool(name="stAw", bufs=2) as paw, \
         tc.tile_pool(name="psA", bufs=1, space="PSUM") as psa:
        xt_i = paw.tile([P, dc, P], BF16, tag="xt", bufs=1)
        nc.sync.dma_start(xt_i[:], dram.xt)
        qr_ps = psa.tile([P, cfg.qlr], F32, tag="qr", bufs=1)
        kv_ps = psa.tile([P, HD], F32, tag="kv", bufs=1)
        ngr = 8
        gsz = dc // ngr
        for gi in range(ngr):
            wqa_g = pa.tile([P, gsz, cfg.qlr], BF16, tag="wqa")
            nc.sync.dma_start(wqa_g[:],
                              dram.wqa[:, gi * gsz:(gi + 1) * gsz, :])
            wkv_g = pa.tile([P, gsz, HD], BF16, tag="wkv")
            nc.sync.dma_start(wkv_g[:],
                              dram.wkv[:, gi * gsz:(gi + 1) * gsz, :])
            for kk in range(gsz):
                k = gi * gsz + kk
                st, sp = k == 0, k == dc - 1
                for n2 in range(cfg.qlr // 512):
                    nc.tensor.matmul(
                        qr_ps[:, n2 * 512:(n2 + 1) * 512],
                        xt_i[:, k, :],
                        wqa_g[:, kk, n2 * 512:(n2 + 1) * 512],
                        start=st, stop=sp)
                nc.tensor.matmul(kv_ps[:], xt_i[:, k, :],
                                 wkv_g[:, kk, :], start=st, stop=sp)

        # qr epilogue: cast + transpose only.  The qr RMS norm is skipped:
        # the per-head q-RMS scale (folded into the softmax exp) cancels
        # any per-row scaling of qr, so normalizing here is a no-op up to
        # eps=1e-6.
        qrn = paw.tile([P, cfg.qlr], BF16, tag="qrn")
        nc.any.tensor_copy(qrn[:], qr_ps[:])
        qrT_loc = paw.tile([P, qc, P], BF16, tag="qrT_loc", bufs=1)
        nc.sync.dma_start_transpose(qrT_loc[:], qrn[:])

        # local kv epilogue (kv_norm_w folded into wkv on the host)
        sqk = paw.tile([P, HD], F32, tag="sqk", bufs=1)
        ssqk = paw.tile([P, 1], F32, tag="ssqk")
        nc.scalar.activation(sqk[:], kv_ps[:], ACTF.Square,
                             accum_out=ssqk[:])
        rinvk = paw.tile([P, 1], F32, tag="rinvk")
        _rsqrt_dve(nc, paw, rinvk[:], ssqk[:], 1.0 / HD, 1.0, tag="rkvn")
        kv_loc = paw.tile([P, HD], BF16, tag="kv_loc", bufs=1)
        nc.scalar.mul(kv_loc[:], kv_ps[:], rinvk[:])
        _rope_tail(nc, paw, kv_loc[:], g.cosm[:], g.sinm[:],
                   False, tag="rkv", bufs=2)

        # pack qrT + local kv into DRAM and all-gather
        gw = qc * P + HD
        with tc.tile_pool(name="ccdram", bufs=1, space="DRAM") as ccd:
            gin = ccd.tile([P, gw], BF16)
            gout = ccd.tile([NCORES, P, gw], BF16, addr_space="Shared")
            nc.sync.dma_start(gin[:, 0:qc * P],
                              qrT_loc[:].rearrange("p c s -> p (c s)"))
            nc.sync.dma_start(gin[:, qc * P:gw], kv_loc[:])
            nc.gpsimd.collective_compute(
                "AllGather", ALU.bypass,
                replica_groups=[list(range(NCORES))],
                ins=[gin[:]], outs=[gout[:]])
            for j in range(NCORES):
                nc.sync.dma_start(
                    qrT_sb[:, :, j * P:(j + 1) * P],
                    gout[j, :, 0:qc * P].rearrange("p (c s) -> p c s", c=qc))
                nc.sync.dma_start(kv_sb[:, j, :], gout[j, :, qc * P:gw])
    return qrT_sb, kv_sb


def _bc_preload(nc, cfg, pew, dram):
    """Head 0/1 weight loads, emitted before the next stage A so they are
    not head-of-line blocked behind its collective on the Pool queue."""
    hc, qc = cfg.hc, cfg.qc
    pre = []
    for h in range(2):
        wqb_h = pew.tile([P, qc, HD], BF16, tag="wqb_h", bufs=2)
        nc.gpsimd.dma_start(wqb_h[:], dram.wqb[:, :, h * HD:(h + 1) * HD])
        woa_h = None
        if h == 0:
            woa_h = pew.tile([P, hc, cfg.olr], BF16, tag="woa_h", bufs=1)
            nc.gpsimd.dma_start(woa_h[:],
                                dram.woa[:, h * hc:(h + 1) * hc, :])
        pre.append((wqb_h, woa_h))
    return pre


def _stage_bc(nc, tc, cfg, g, ogp, pew, iface, dram, pre):
    """Per-head q proj + attention + wo_a partials.  Returns (ogT, wob_qs)
    for stage E."""
    sc, qc, hc = cfg.sc, cfg.qc, cfg.hc
    qrT_sb, kv_sb = iface
    kvT_sb = g.kvT
    ident, cmask = g.ident, g.cmask
    cos_sb, sin_sb, sinkexp_sb = g.cos, g.sin, g.sinkexp
    s_chunks = [(a, min(512, cfg.s - a)) for a in range(0, cfg.s, 512)]

    og_acc = ogp.tile([P, cfg.oc, cfg.s], F32, tag="og", bufs=1)
    ogT_sb = ogp.tile([P, cfg.oc, cfg.s], BF16, tag="ogT", bufs=1)
    # prefetch the first wo_b eighths (bufs=3 so these never WAR-wait);
    # the rest stream inside stage E right before use
    wob_qs = []
    for e in range(3):
        wob_q = pew.tile([P, cfg.oc, 512], BF16, tag="wobq", bufs=3)
        nc.scalar.dma_start(wob_q[:],
                            dram.wob[:, :, e * 512:(e + 1) * 512])
        wob_qs.append(wob_q)

    # kvT derived from the gathered kv (xbar transpose)
    for j in range(sc):
        nc.sync.dma_start_transpose(
            kvT_sb[:, :, j * P:(j + 1) * P], kv_sb[:, j, :])

    with tc.tile_pool(name="stBC", bufs=1) as pb, \
         tc.tile_pool(name="stBCw", bufs=2) as pbw, \
         tc.tile_pool(name="psQ", bufs=1, space="PSUM") as psq, \
         tc.tile_pool(name="psS", bufs=1, space="PSUM") as pss, \
         tc.tile_pool(name="psT", bufs=1, space="PSUM") as pst, \
         tc.tile_pool(name="psO", bufs=1, space="PSUM") as pso, \
         tc.tile_pool(name="psD", bufs=1, space="PSUM") as psd:
        for h in range(cfg.hpc):
            if h < 2 and pre[h][1] is not None:
                wqb_h, woa_h = pre[h]
            elif h < 2:
                wqb_h = pre[h][0]
                woa_h = pew.tile([P, hc, cfg.olr], BF16, tag="woa_h",
                                 bufs=1)
                nc.gpsimd.dma_start(
                    woa_h[:], dram.woa[:, h * hc:(h + 1) * hc, :])
            else:
                woa_h = pew.tile([P, hc, cfg.olr], BF16, tag="woa_h",
                                 bufs=1)
                nc.gpsimd.dma_start(
                    woa_h[:], dram.woa[:, h * hc:(h + 1) * hc, :])
                wqb_h = pew.tile([P, qc, HD], BF16, tag="wqb_h", bufs=2)
                nc.gpsimd.dma_start(
                    wqb_h[:], dram.wqb[:, :, h * HD:(h + 1) * HD])
            qT_sb = pbw.tile([P, hc, cfg.s], BF16, tag="qT", bufs=1)
            # ---- q projection (q left unnormalized; the RMS scale is
            # folded into the softmax exp below) ----
            q8 = pbw.tile([P, sc, HD], BF16, tag="q8", bufs=1)
            ssq8 = pbw.tile([P, sc], F32, tag="ssq8")
            for i in range(sc):
                q_ps = psq.tile([P, HD], F32, tag="q", bufs=2)
                for c in range(qc):
                    nc.tensor.matmul(
                        q_ps[:], qrT_sb[:, c, i * P:(i + 1) * P],
                        wqb_h[:, c, :], start=(c == 0), stop=(c == qc - 1))
                nc.any.tensor_copy(q8[:, i, :], q_ps[:])
                sqq = pbw.tile([P, HD], F32, tag="sqq", bufs=1)
                nc.scalar.activation(sqq[:], q8[:, i, :], ACTF.Square,
                                     accum_out=ssq8[:, i:i + 1])
            rinv8 = pbw.tile([P, sc], F32, tag="rinv8")
            _rsqrt_dve(nc, pbw, rinv8[:], ssq8[:], 1.0 / HD,
                       HD ** -0.5, tag="rq8")
            _rope_tail(nc, pbw, q8[:], cos_sb[:], sin_sb[:],
                       False, tag="rq")
            for i in range(sc):
                tpq = pst.tile([P, 512], BF16, tag="t", bufs=1)
                for c in range(hc):
                    nc.tensor.transpose(tpq[:, c * P:(c + 1) * P],
                                        q8[:, i, c * P:(c + 1) * P],
                                        ident[:])
                nc.any.tensor_copy(
                    qT_sb[:, :, i * P:(i + 1) * P],
                    tpq[:].rearrange("p (c s) -> p c s", c=hc))

            # ---- attention for head h (max-free softmax) ----
            o8 = pbw.tile([P, sc, HD], BF16, tag="o8", bufs=1)
            for i in range(sc):
                w_all = (i + 1) * P
                nch = (w_all + 511) // 512
                s_ps = []
                for ci in range(nch):
                    wci = min(512, w_all - ci * 512)
                    s_ps.append((pss.tile([P, 512], F32, tag="s",
                                          bufs=2, name="s_ps"), wci))
                for k in range(hc):
                    for ci in range(nch):
                        tile_ps, wci = s_ps[ci]
                        nc.tensor.matmul(
                            tile_ps[:, :wci],
                            qT_sb[:, k, i * P:(i + 1) * P],
                            kvT_sb[:, k, ci * 512:ci * 512 + wci],
                            start=(k == 0), stop=(k == hc - 1))
                # causal mask on the diagonal block
                dps, dw = s_ps[-1]
                dcol = (w_all - P) - (nch - 1) * 512
                nc.vector.tensor_add(dps[:, dcol:dcol + P],
                                     dps[:, dcol:dcol + P], cmask[:])
                # exp with the q-RMS scale folded in; row sums accumulate
                # per chunk (no max subtraction: |logit| <= sqrt(HD) so
                # exp stays in f32 range)
                p_sb = pbw.tile([P, cfg.s], BF16, tag="p")
                l0 = pb.tile([P, 2], F32, tag="l0", bufs=2)
                for ci in range(nch):
                    tile_ps, wci = s_ps[ci]
                    nc.scalar.activation(
                        p_sb[:, ci * 512:ci * 512 + wci],
                        tile_ps[:, :wci], ACTF.Exp,
                        scale=rinv8[:, i:i + 1],
                        accum_out=l0[:, ci:ci + 1])
                lsum = pb.tile([P, 1], F32, tag="lsum", bufs=2)
                if nch == 1:
                    nc.vector.tensor_add(lsum[:], l0[:, 0:1],
                                         sinkexp_sb[:, h:h + 1])
                else:
                    nc.vector.tensor_add(lsum[:], l0[:, 0:1], l0[:, 1:2])
                    nc.vector.tensor_add(lsum[:], lsum[:],
                                         sinkexp_sb[:, h:h + 1])
                linv = pb.tile([P, 1], F32, tag="linv", bufs=2)
                nc.vector.reciprocal(linv[:], lsum[:])
                # transpose p
                pT_sb = pbw.tile([P, cfg.s], BF16, tag="pT")
                for gi in range((i + 1 + 3) // 4):
                    jn = min(4, (i + 1) - gi * 4)
                    tpp = pst.tile([P, 512], BF16, tag="t", bufs=1)
                    for j4 in range(jn):
                        j = gi * 4 + j4
                        nc.tensor.transpose(
                            tpp[:, j4 * P:(j4 + 1) * P],
                            p_sb[:, j * P:(j + 1) * P], ident[:])
                    nc.any.tensor_copy(
                        pT_sb[:, gi * 512:gi * 512 + jn * P],
                        tpp[:, :jn * P])
                # o = p^T-weighted sum of kv rows
                o_ps = pso.tile([P, HD], F32, tag="o", bufs=1)
                for j in range(i + 1):
                    nc.tensor.matmul(o_ps[:], pT_sb[:, j * P:(j + 1) * P],
                                     kv_sb[:, j, :],
                                     start=(j == 0), stop=(j == i))
                # normalize into o8 (batched inv-rope later)
                nc.vector.tensor_scalar_mul(o8[:, i, :], o_ps[:], linv[:])

            _rope_tail(nc, pbw, o8[:], cos_sb[:], sin_sb[:], True, tag="ro")
            oT_h = pbw.tile([P, hc, cfg.s], BF16, tag="oT_h", bufs=1)
            for i in range(sc):
                tpo = pst.tile([P, 512], BF16, tag="t", bufs=1)
                for c in range(hc):
                    nc.tensor.transpose(tpo[:, c * P:(c + 1) * P],
                                        o8[:, i, c * P:(c + 1) * P],
                                        ident[:])
                nc.any.tensor_copy(
                    oT_h[:, :, i * P:(i + 1) * P],
                    tpo[:].rearrange("p (c s) -> p c s", c=hc))

            # ---- wo_a partial for this head, into f32 og_acc ----
            for m in range(cfg.oc):
                d_ps = []
                for n2 in range(len(s_chunks)):
                    d_ps.append(psd.tile([P, 512], F32, tag=f"d{n2}",
                                         bufs=1, name="d_ps"))
                for kk in range(hc):
                    for n2, (a, w) in enumerate(s_chunks):
                        nc.tensor.matmul(
                            d_ps[n2][:, :w],
                            woa_h[:, kk, m * P:(m + 1) * P],
                            oT_h[:, kk, a:a + w],
                            start=(kk == 0), stop=(kk == hc - 1))
                for n2, (a, w) in enumerate(s_chunks):
                    if h == 0:
                        nc.vector.tensor_copy(og_acc[:, m, a:a + w],
                                              d_ps[n2][:, :w])
                    elif h == cfg.hpc - 1:
                        # final add writes bf16 ogT directly
                        nc.vector.tensor_add(ogT_sb[:, m, a:a + w],
                                             og_acc[:, m, a:a + w],
                                             d_ps[n2][:, :w])
                    else:
                        nc.vector.tensor_add(og_acc[:, m, a:a + w],
                                             og_acc[:, m, a:a + w],
                                             d_ps[n2][:, :w])
    return ogT_sb, wob_qs


def _stage_e(nc, tc, cfg, pew, ogT_sb, wob_qs, dram):
    """Final wo_b partial matmul, streaming output to DRAM."""
    sc = cfg.sc
    with tc.tile_pool(name="psE", bufs=1, space="PSUM") as pse:
        for quarter in range(cfg.nc_out):
            if quarter < len(wob_qs):
                wob_q = wob_qs[quarter]
            else:
                wob_q = pew.tile([P, cfg.oc, 512], BF16, tag="wobq",
                                 bufs=3)
                nc.scalar.dma_start(
                    wob_q[:],
                    dram.wob[:, :, quarter * 512:(quarter + 1) * 512])
            for m in range(sc):
                out_ps = pse.tile([P, 512], F32, tag="out", bufs=8,
                                  name="out_ps")
                for k in range(cfg.oc):
                    nc.tensor.matmul(
                        out_ps[:], ogT_sb[:, k, m * P:(m + 1) * P],
                        wob_q[:, k, :], start=(k == 0),
                        stop=(k == cfg.oc - 1))
                o_out = pew.tile([P, 512], F32, tag="oo", bufs=3)
                nc.vector.tensor_copy(o_out[:], out_ps[:])
                nc.scalar.dma_start(
                    dram.out[m, :, quarter * 512:(quarter + 1) * 512],
                    o_out[:])


def build_program(cfg: Cfg, debug=False, reps=1):
    nc = bacc.Bacc("TRN2", debug=False, num_devices=NCORES)
    dram = _Dram(nc, cfg)

    with tile.TileContext(nc) as tc:
        with tc.tile_pool(name="glob", bufs=1) as gp, \
             tc.tile_pool(name="og", bufs=1) as ogp, \
             tc.tile_pool(name="stEw", bufs=2) as pew:
            g = _Glob(nc, gp, cfg, dram)
            # software pipeline: stage A of rep r runs between BC and E of
            # rep r-1, hiding the all-gather under stage E
            prev = _stage_a(nc, tc, cfg, gp, g, dram)
            for _ in range(reps - 1):
                pre = _bc_preload(nc, cfg, pew, dram)
                nxt = _stage_a(nc, tc, cfg, gp, g, dram)
                bce = _stage_bc(nc, tc, cfg, g, ogp, pew, prev, dram, pre)
                _stage_e(nc, tc, cfg, pew, bce[0], bce[1], dram)
                prev = nxt
            pre = _bc_preload(nc, cfg, pew, dram)
            bce = _stage_bc(nc, tc, cfg, g, ogp, pew, prev, dram, pre)
            _stage_e(nc, tc, cfg, pew, bce[0], bce[1], dram)

    nc.compile()
    return nc


# ---------------------------------------------------------------------------
# host side
# ---------------------------------------------------------------------------

def _pack_kt(w, n_rows, n_cols):
    """Pack W (given as [n_cols, n_rows] np array) into [128, n_rows/128,
    n_cols] = W.T tiled with the contraction dim on partitions."""
    wt = np.ascontiguousarray(w.T)  # [n_rows, n_cols]
    return np.ascontiguousarray(
        wt.reshape(n_rows // P, P, n_cols).transpose(1, 0, 2))


def prepare_inmaps(inputs, cfg: Cfg):
    bf = NPBF16
    x = np.asarray(inputs["x"], dtype=bf).reshape(cfg.s, cfg.d)
    xt = np.ascontiguousarray(
        x.T.reshape(cfg.dc, P, cfg.sc, P).transpose(2, 1, 0, 3))

    wq_a = np.asarray(inputs["wq_a"], dtype=bf)
    wqa = _pack_kt(wq_a, cfg.d, cfg.qlr)

    kv_norm_w = np.asarray(inputs["kv_norm_w"], dtype=np.float32)
    wkv_f = (np.asarray(inputs["wkv"], dtype=bf).astype(np.float32)
             * kv_norm_w[:, None]).astype(bf)
    wkv = _pack_kt(wkv_f, cfg.d, HD)

    q_norm_w = np.asarray(inputs["q_norm_w"], dtype=np.float32)
    wq_b = np.asarray(inputs["wq_b"], dtype=bf).astype(np.float32)
    wq_b = (wq_b * q_norm_w[None, :]).astype(bf)  # fold q_norm into wq_b

    cos = np.asarray(inputs["cos"], dtype=np.float32)
    sin = np.asarray(inputs["sin"], dtype=np.float32)
    cos_p = np.ascontiguousarray(
        cos.reshape(cfg.sc, P, RD // 2).transpose(1, 0, 2))
    sin_p = np.ascontiguousarray(
        sin.reshape(cfg.sc, P, RD // 2).transpose(1, 0, 2))

    wo_a = np.asarray(inputs["wo_a"], dtype=bf)  # [OG*OLR, F]
    wo_b = np.asarray(inputs["wo_b"], dtype=bf)  # [D, OG*OLR]
    sink = np.asarray(inputs["attn_sink"], dtype=np.float32)

    in_maps = []
    for c in range(NCORES):
        h0 = c * cfg.hpc
        wqb_c = wq_b[h0 * HD:(h0 + cfg.hpc) * HD, :]  # [hpc*HD, qlr]
        woa_c = wo_a[c * cfg.olr:(c + 1) * cfg.olr, :]  # [olr, F]
        wob_c = wo_b[:, c * cfg.olr:(c + 1) * cfg.olr]  # [outd, olr]
        sinkexp_c = np.exp(sink[h0:h0 + cfg.hpc])
        in_maps.append({
            "xtm": np.ascontiguousarray(xt[c]),
            "cosm": np.ascontiguousarray(cos_p[:, c, :]),
            "sinm": np.ascontiguousarray(sin_p[:, c, :]),
            "wqa": wqa,
            "wkv": wkv,
            "wqb": _pack_kt(wqb_c, cfg.qlr, cfg.hpc * HD),
            "woa": _pack_kt(woa_c, cfg.f, cfg.olr),
            "wob": _pack_kt(wob_c, cfg.olr, cfg.outd),
            "coss": cos_p,
            "sins": sin_p,
            "sinkexp": np.ascontiguousarray(
                np.broadcast_to(sinkexp_c, (P, cfg.hpc))).astype(np.float32),
        })
    return in_maps


_CACHE = {}


def _get_program():
    if "nc" not in _CACHE:
        _CACHE["nc"] = build_program(Cfg())
    return _CACHE["nc"]


def run(inputs, trace=False):
    """Returns (output [1,S,D] bf16, BassKernelResults)."""
    cfg = Cfg()
    nc = _get_program()
    in_maps = prepare_inmaps(inputs, cfg)
    res = run_bass_kernel_spmd(nc, in_maps, core_ids=list(range(NCORES)),
                               trace=trace)
    acc = np.zeros((cfg.s, cfg.outd), np.float32)
    for r in res.results:
        acc += r["out"].reshape(cfg.s, cfg.outd)
    out = acc.astype(NPBF16).reshape(1, cfg.s, cfg.outd)
    return out, res


def kernel(**inputs) -> np.ndarray:
    out, _ = run(inputs)
    return out
